# revision 47
# baseline (speedup 1.0000x reference)
"""Trainium2 Bass kernel for nn_BaselineModelWithGNN (8-core SPMD).

Self-contained: hardcodes shapes/sharding; builds, compiles and runs the Bass
program on 8 NeuronCores via the axon PJRT path.

Key observation: the reference applies each of the 3 GCN convs to the same
input x and overwrites `out`, so only conv i=2 (w_conv[2], b_conv[2],
bng[4]) affects the result — one conv is computed.

Sharding: nodes (and their incident edges, dst-sharded) are partitioned
contiguously across the 8 cores (4096 nodes / 8 graphs per core); the
PLM/pooling branch is data-parallel over batch.

Optimizations vs the naive mapping:
- BatchNorm folding: BN is affine given its batch stats, so each BN is folded
  into the NEXT linear layer's weights/bias on device (W' = diag(g/s)W,
  b' = t@W + b_next). No full-tensor BN application passes.
- The GCN aggregates RAW y = dinv*relu(z2) (pre-BN2); the BN2 affine
  correction commutes through the (linear) aggregation:
    conv_pre[d] = dinv_d*(A[d] @ Wc') + u_d*(t2@Wc) + b_conv
  with A[d] = sum_src y[src], u_d = dinv_d*S1[d], S1[d] = sum_src dinv[src]
  (host-precomputed from degrees). So the edge-gather phase does not wait on
  the BN2 stats sync.
- W_conv is applied AFTER aggregation (they commute), removing the z matmul
  from the pre-collective critical path.
- Self-loop contributions are seeded into PSUM via an identity matmul
  instead of gathered (self-loops dropped from the edge list).
- BN batch stats are synced with AllGather + local sum (cheaper than
  AllReduce in the profile model); stat sums come free from the Activation
  engine's accum_out during the relu pass.
- The sentence branch is emitted after the y AllGather so its DMA/PE work
  hides under the collective.
"""
import sys
sys.path.insert(0, "/opt/trn_rl_repo")
from contextlib import ExitStack

import numpy as np
import ml_dtypes

import bass_rust as _br
import concourse.bacc as bacc
import concourse.bass as bass
import concourse.tile as tile
from concourse import mybir
from concourse._compat import cdiv

fp32 = mybir.dt.float32
bf16 = mybir.dt.float16  # fp16 for 8x less quant noise than bf16
i16 = mybir.dt.int16
AF = mybir.ActivationFunctionType
ALU = mybir.AluOpType
AX = mybir.AxisListType

NCORES = 8
B, S, E = 64, 512, 768
D = 384
NG = 512
N = B * NG              # 32768
NEDGE = 1048576
C = 3
NPC = N // NCORES       # 4096 nodes per core
GPC = B // NCORES       # 8 graphs per core
NW = NPC // 128         # 32 dst windows per core
NCOL = 512
NCH = NPC // NCOL       # 8 column chunks
EPS = 1e-5
GCHUNK = 12             # gather chunk: tiles (of 128 edges) per dma_gather


# ---------------------------------------------------------------- BIR patch
def split_waits(nc):
    """walrus here supports ONE sync-wait per instruction; split extras onto
    NoOps inserted just before, on the same engine."""
    counter = 0
    for f in nc.m.functions:
        for bb in f.blocks:
            newlist, changed = [], False
            for inst in bb.instructions:
                si = inst.sync_info
                if si is not None and len(si.on_wait) > 1:
                    waits = list(si.on_wait)
                    for w in waits[:-1]:
                        counter += 1
                        nop = mybir.InstNoOp(name=f"I-WSPLIT-{counter}", ins=[], outs=[])
                        nop.engine = inst.engine
                        nop.sync_info = _br.SyncInfo(on_wait=[w], on_update=[])
                        newlist.append(nop)
                    inst.sync_info = _br.SyncInfo(
                        on_wait=[waits[-1]], on_update=list(si.on_update))
                    changed = True
                newlist.append(inst)
            if changed:
                bb.instructions = newlist


# ---------------------------------------------------------------- host prep
def _col3(v):
    """[384] -> [128, 3] column layout (feature f = c*128+p)."""
    return np.ascontiguousarray(np.asarray(v).reshape(3, 128).T).astype(np.float32)


def _col6(v):
    return np.ascontiguousarray(np.asarray(v).reshape(6, 128).T).astype(np.float32)


def _wchunks(w, kc, m):
    """[K, M] -> [128, kc, M] (k-chunk on partitions)."""
    K, M = w.shape
    assert K == kc * 128
    return np.ascontiguousarray(w.reshape(kc, 128, M).transpose(1, 0, 2))


def _wrap_idx(idx):
    """int16 idx array (len % 128 == 0) -> [128, len/16] dma_gather layout."""
    blk = idx.reshape(-1, 16).T  # [16, len/16]
    return np.ascontiguousarray(np.tile(blk, (8, 1)))


def preprocess(inputs):
    ei = np.asarray(inputs["edge_index"]).astype(np.int64)
    loop = np.arange(N, dtype=np.int64)
    dst_all = np.concatenate([ei[1], loop])

    deg = np.bincount(dst_all, minlength=N).astype(np.float32)
    dinv = 1.0 / np.sqrt(deg)
    # S1[d] = sum over in-edges (incl self-loop) of dinv[src]; u = dinv*S1
    S1 = np.bincount(ei[1], weights=dinv[ei[0]], minlength=N) + dinv
    u = (dinv * S1).astype(np.float32)

    # self-loops handled on device (PSUM identity seed): edges only
    src_e = ei[0]
    dst_e = ei[1]
    win = dst_e >> 7
    order = np.lexsort((src_e, win))
    src_s = src_e[order]
    dst_s = dst_e[order]

    wcnt = np.bincount(win, minlength=N // 128)          # [256]
    tpw = np.maximum(
        np.ceil(wcnt.reshape(NCORES, NW) / 128).max(axis=0), 1
    ).astype(np.int64)                                    # [32] shared schedule
    T_total = int(tpw.sum())
    wstart = np.zeros(N // 128 + 1, np.int64)
    np.cumsum(wcnt, out=wstart[1:])
    tstart = np.zeros(NW + 1, np.int64)
    np.cumsum(tpw, out=tstart[1:])

    # masked node indices (2 per graph, ascending)
    mask = np.asarray(inputs["graph_masking"])
    sel = np.argsort(-mask, axis=1, kind="stable")[:, :2]  # top_k: ones, asc idx
    sel = np.sort(sel, axis=1)

    xT = np.zeros((D, N), np.float32)
    xT[:300] = np.asarray(inputs["x_nodes"]).T
    xT = xT.astype(np.float16)

    w1p = np.zeros((D, D), np.float32)
    w1p[:300] = np.asarray(inputs["w_pre1"])

    lastf = np.asarray(inputs["last_h"]).astype(np.float16)
    firstf = np.asarray(inputs["first_h"]).astype(np.float16)

    bng_g, bng_b = np.asarray(inputs["bng_g"]), np.asarray(inputs["bng_b"])
    bn_g, bn_b = np.asarray(inputs["bn_g"]), np.asarray(inputs["bn_b"])
    # vec columns [128, 75]: order documented here, mirrored on device
    cols = [
        _col3(inputs["b_pre1"]), _col3(inputs["b_pre2"]),
        _col3(inputs["b_post1"]), _col3(inputs["b_post2"]),
        _col3(np.asarray(inputs["b_conv"])[2]),
        _col6(inputs["b_cat"]),
        _col3(bng_g[0]), _col3(bng_b[0]), _col3(bng_g[1]), _col3(bng_b[1]),
        _col3(bng_g[4]), _col3(bng_b[4]), _col3(bng_g[5]), _col3(bng_b[5]),
        _col3(bng_g[6]), _col3(bng_b[6]),
        _col6(bn_g[0]), _col6(bn_b[0]), _col6(bn_g[1]), _col6(bn_b[1]),
    ]
    vecs = np.concatenate(cols, axis=1)  # [128, 3*5+6+3*10+6*4] = [128, 75]
    brow = np.zeros((1, 512), np.float32)
    brow[0, D:D + C] = np.asarray(inputs["b_out"])

    w_bf = {
        "w1": _wchunks(w1p, 3, D).astype(np.float16),
        "w2": _wchunks(np.asarray(inputs["w_pre2"]), 3, D).astype(np.float16),
        "wc": _wchunks(np.asarray(inputs["w_conv"])[2], 3, D).astype(np.float16),
        "wp1": _wchunks(np.asarray(inputs["w_post1"]), 3, D).astype(np.float16),
        "wp2": _wchunks(np.asarray(inputs["w_post2"]), 3, D).astype(np.float16),
    }
    wcat = _wchunks(np.asarray(inputs["w_cat"]), 6, E).astype(np.float16)
    wout = _wchunks(np.asarray(inputs["w_out"]), 6, C).astype(np.float32)

    in_maps = []
    for c in range(NCORES):
        n0 = c * NPC
        src_pad = np.zeros(T_total * 128, np.int64)
        dstloc = np.full(T_total * 128, -1.0, np.float32)
        for w in range(NW):
            gw = c * NW + w
            a, b_ = wstart[gw], wstart[gw + 1]
            k = b_ - a
            pos = tstart[w] * 128
            src_pad[pos:pos + k] = src_s[a:b_]
            dstloc[pos:pos + k] = (dst_s[a:b_] - gw * 128).astype(np.float32)
        # int16: N-1 = 32767 fits exactly
        idx_w = _wrap_idx(src_pad.astype(np.int16))            # [128, T*8]
        dst_t = np.ascontiguousarray(dstloc.reshape(T_total, 128).T)  # [128, T]

        deg_nm = np.ascontiguousarray(
            deg[n0:n0 + NPC].reshape(NW, 128).T)               # [128, 32]
        u_row = u[n0:n0 + NPC].reshape(1, NPC)                 # [1, 4096]

        gidx = (sel[c * GPC:(c + 1) * GPC] +
                np.arange(c * GPC, (c + 1) * GPC)[:, None] * NG - n0)  # local
        gidx = gidx.reshape(-1).astype(np.int16)               # [16]
        gidx_w = np.zeros((128, 1), np.int16)
        gidx_w[:16, 0] = gidx
        gidx_w = np.tile(gidx_w[:16], (8, 1))

        m = {
            "lasth": np.ascontiguousarray(
                lastf[c * GPC:(c + 1) * GPC].reshape(GPC * S, E)),
            "firsth": np.ascontiguousarray(
                firstf[c * GPC:(c + 1) * GPC].reshape(GPC * S, E)),
            "xT": np.ascontiguousarray(
                xT.reshape(3, 128, N)[:, :, n0:n0 + NPC].transpose(1, 0, 2)
            ).reshape(128, 3 * NPC),
            "eidx": idx_w, "dstloc": dst_t, "deg": deg_nm, "urow": u_row,
            "vecs": vecs, "brow": brow, "gidx": gidx_w,
            "w1": w_bf["w1"].reshape(128, 3 * D),
            "w2": w_bf["w2"].reshape(128, 3 * D),
            "wc": w_bf["wc"].reshape(128, 3 * D),
            "wp1": w_bf["wp1"].reshape(128, 3 * D),
            "wp2": w_bf["wp2"].reshape(128, 3 * D),
            "wcat": wcat.reshape(128, 6 * E),
            "wout": wout.reshape(128, 6 * C),
        }
        in_maps.append(m)
    meta = (tuple(int(t) for t in tpw),)
    return in_maps, meta


# ---------------------------------------------------------------- device
def build(meta, rep=1, taps=(), stage=99):
    tpw = meta[0]
    T_total = sum(tpw)
    tstart = [0]
    for t in tpw:
        tstart.append(tstart[-1] + t)

    nc = bacc.Bacc("TRN2", dynamic_dma_scratch_size=65536)
    I = {}
    I["lasth"] = nc.dram_tensor("lasth", [GPC * S, E], bf16, kind="ExternalInput")
    I["firsth"] = nc.dram_tensor("firsth", [GPC * S, E], bf16, kind="ExternalInput")
    I["xT"] = nc.dram_tensor("xT", [128, 3 * NPC], bf16, kind="ExternalInput")
    I["eidx"] = nc.dram_tensor("eidx", [128, T_total * 8], i16, kind="ExternalInput")
    I["dstloc"] = nc.dram_tensor("dstloc", [128, T_total], fp32, kind="ExternalInput")
    I["deg"] = nc.dram_tensor("deg", [128, NW], fp32, kind="ExternalInput")
    I["urow"] = nc.dram_tensor("urow", [1, NPC], fp32, kind="ExternalInput")
    I["vecs"] = nc.dram_tensor("vecs", [128, 75], fp32, kind="ExternalInput")
    I["brow"] = nc.dram_tensor("brow", [1, 512], fp32, kind="ExternalInput")
    I["gidx"] = nc.dram_tensor("gidx", [128, 1], i16, kind="ExternalInput")
    for w in ("w1", "w2", "wc", "wp1", "wp2"):
        I[w] = nc.dram_tensor(w, [128, 3 * D], bf16, kind="ExternalInput")
    I["wcat"] = nc.dram_tensor("wcat", [128, 6 * E], bf16, kind="ExternalInput")
    I["wout"] = nc.dram_tensor("wout", [128, 6 * C], fp32, kind="ExternalInput")
    outT = nc.dram_tensor("outT", [C, GPC], fp32, kind="ExternalOutput")
    tap_outs = {}

    grp = [list(range(NCORES))]

    with tile.TileContext(nc) as tc, ExitStack() as ctx:
        const = ctx.enter_context(tc.tile_pool(name="const", bufs=1))
        big = ctx.enter_context(tc.tile_pool(name="big", bufs=2))
        gpool = ctx.enter_context(tc.tile_pool(name="gath", bufs=3))
        spool = ctx.enter_context(tc.tile_pool(name="small", bufs=2))
        selp = ctx.enter_context(tc.tile_pool(name="sel", bufs=8))
        hspool = ctx.enter_context(tc.tile_pool(name="hs", bufs=8))
        sqpool = ctx.enter_context(tc.tile_pool(name="sq", bufs=2))
        mmps = ctx.enter_context(tc.tile_pool(name="mmps", bufs=2, space="PSUM"))
        cvps = ctx.enter_context(tc.tile_pool(name="cvps", bufs=2, space="PSUM"))
        trps = ctx.enter_context(tc.tile_pool(name="trps", bufs=2, space="PSUM"))
        typs = ctx.enter_context(tc.tile_pool(name="typs", bufs=1, space="PSUM"))
        tfps = ctx.enter_context(tc.tile_pool(name="tfps", bufs=1, space="PSUM"))
        dram = ctx.enter_context(tc.tile_pool(name="dram", bufs=1, space="DRAM"))

        # ---------------- constants
        iota = const.tile([128, 128], fp32)
        nc.gpsimd.iota(iota[:], pattern=[[1, 128]], base=0, channel_multiplier=0,
                       allow_small_or_imprecise_dtypes=True)
        pidx = const.tile([128, 1], fp32)  # partition index column
        nc.gpsimd.iota(pidx[:], pattern=[[0, 1]], base=0, channel_multiplier=1,
                       allow_small_or_imprecise_dtypes=True)
        ident = const.tile([128, 128], bf16)
        nc.vector.tensor_scalar(ident[:], iota[:], pidx[:], None, ALU.is_equal)
        identf = const.tile([128, 128], fp32)
        nc.vector.tensor_scalar(identf[:], iota[:], pidx[:], None, ALU.is_equal)

        # head-critical loads only; gather-phase data (idx/dst/u/gidx) and
        # tail data (wcat/wout/brow) are DMA'd after the AllGather is issued.
        # xT first: it gates L1, everything else here is small.
        xT_t = big.tile([128, 3, NPC], bf16, tag="big")
        nc.sync.dma_start(xT_t[:], I["xT"][:].rearrange("p (k n) -> p k n", k=3))
        deg_t = const.tile([128, NW], fp32)
        nc.sync.dma_start(deg_t[:], I["deg"][:])
        vecs = const.tile([128, 75], fp32)
        nc.sync.dma_start(vecs[:], I["vecs"][:])
        W = {}
        for w in ("w1", "w2", "wc", "wp1", "wp2"):
            W[w] = const.tile([128, 3, D], bf16, name=f"W_{w}", tag=f"W_{w}")
            nc.sync.dma_start(W[w][:], I[w][:].rearrange("p (k m) -> p k m", k=3))
        idx_t = const.tile([128, T_total * 8], i16)
        dst_t = const.tile([128, T_total], fp32)
        u_row = const.tile([1, NPC], fp32)
        gidx_t = const.tile([128, 1], i16)
        wcat = const.tile([128, 6, E], bf16)
        wout = const.tile([128, 6, C], fp32)
        brow = const.tile([1, 512], fp32)

        def load_deferred(gate_ap):
            """Gate the deferred loads behind `gate_ap` (a WAW dummy write into
            each destination) so the tile scheduler cannot front-load their
            DMA transfers into the head where they'd steal DMA bandwidth."""
            gate32 = const.tile([1, 1], fp32)
            nc.vector.tensor_copy(gate32[:], gate_ap)
            for ap in (idx_t[0:1, 0:1], dst_t[0:1, 0:1], u_row[0:1, 0:1],
                       gidx_t[0:1, 0:1], wcat[0:1, 0, 0:1], wout[0:1, 0, 0:1],
                       brow[0:1, 0:1]):
                nc.vector.tensor_copy(ap, gate32[:])
            nc.sync.dma_start(idx_t[:], I["eidx"][:])
            nc.sync.dma_start(dst_t[:], I["dstloc"][:])
            nc.sync.dma_start(u_row[:], I["urow"][:])
            nc.sync.dma_start(gidx_t[:], I["gidx"][:])
            nc.sync.dma_start(wcat[:], I["wcat"][:].rearrange("p (k m) -> p k m", k=6))
            nc.sync.dma_start(wout[:], I["wout"][:].rearrange("p (k m) -> p k m", k=6))
            nc.sync.dma_start(brow[:], I["brow"][:])

        ones8 = const.tile([1, GPC], fp32)
        nc.vector.memset(ones8[:], 1.0)
        onescol = const.tile([128, 1], bf16)
        nc.vector.memset(onescol[:], 1.0)

        # vec column offsets
        VO = {}
        off = 0
        for name, w_ in [("b1", 3), ("b2", 3), ("bp1", 3), ("bp2", 3),
                         ("bcv", 3), ("bcat", 6),
                         ("g0", 3), ("be0", 3), ("g1", 3), ("be1", 3),
                         ("g4", 3), ("be4", 3), ("g5", 3), ("be5", 3),
                         ("g6", 3), ("be6", 3),
                         ("gc0", 6), ("bc0", 6), ("gc1", 6), ("bc1", 6)]:
            VO[name] = (off, w_)
            off += w_
        def vcol(name):
            o, w_ = VO[name]
            return vecs[:, o:o + w_]

        # deg^-1/2
        dinv = const.tile([128, NW], fp32)
        nc.scalar.sqrt(dinv[:], deg_t[:])
        nc.vector.reciprocal(dinv[:], dinv[:])

        def tap(name, ap):
            if name not in taps:
                return
            t_ = nc.dram_tensor(f"tap_{name}", list(ap.shape), ap.dtype,
                                kind="ExternalOutput")
            tap_outs[name] = t_
            nc.sync.dma_start(t_[:], ap)

        def finish_early(src_ap, width=GPC):
            fin0 = spool.tile([C, GPC], fp32, name="fin0", tag="fin")
            nc.vector.memset(fin0[:], 0.0)
            nc.vector.tensor_scalar(fin0[:, 0:width], src_ap, 1.0, None, ALU.mult)
            nc.sync.dma_start(outT[:], fin0[:])

        for _rep in range(rep):
            # ---------------- helpers
            def gather_stats(st, width, tag):
                """st [128, width] per-core sums -> AllGather + local sum."""
                cin = dram.tile([128, width], fp32, tag=tag + "_ci")
                cout = dram.tile([NCORES * 128, width], fp32, tag=tag + "_co")
                nc.sync.dma_start(cin[:], st)
                nc.gpsimd.collective_compute(
                    "AllGather", ALU.bypass, replica_groups=grp,
                    ins=[cin[:]], outs=[cout[:]])
                allst = spool.tile([128, NCORES, width], fp32, tag=tag + "_as")
                nc.sync.dma_start(
                    allst[:], cout[:].rearrange("(g p) c -> p g c", g=NCORES))
                av = allst[:].rearrange("p g c -> p (g c)")
                red = spool.tile([128, width], fp32, tag=tag + "_rd")
                half = spool.tile([128, 4 * width], fp32, tag=tag + "_hf")
                nc.vector.tensor_tensor(
                    half[:], av[:, 0:4 * width], av[:, 4 * width:8 * width], ALU.add)
                nc.vector.tensor_tensor(
                    half[:, 0:2 * width], half[:, 0:2 * width],
                    half[:, 2 * width:4 * width], ALU.add)
                nc.vector.tensor_tensor(
                    red[:], half[:, 0:width], half[:, width:2 * width], ALU.add)
                return red

            def bn_coeffs(red, m_chunks, count, gname, bname, tag):
                """red [128, 2*m]: per-feature sum/sumsq -> gp, bp [128, m]
                gp = g/sigma, bp = b - mu*g/sigma."""
                gp = spool.tile([128, m_chunks], fp32, tag=tag + "_gp")
                bp = spool.tile([128, m_chunks], fp32, tag=tag + "_bp")
                mu = spool.tile([128, m_chunks], fp32, tag=tag + "_mu")
                var = spool.tile([128, m_chunks], fp32, tag=tag + "_va")
                inv_n = 1.0 / count
                sview = red.rearrange("p (m two) -> p m two", two=2)
                nc.vector.tensor_scalar(mu[:], sview[:, :, 0], inv_n, None, ALU.mult)
                nc.vector.tensor_scalar(var[:], sview[:, :, 1], inv_n, None, ALU.mult)
                musq = spool.tile([128, m_chunks], fp32, tag=tag + "_ms")
                nc.vector.tensor_mul(musq[:], mu[:], mu[:])
                nc.vector.tensor_tensor(var[:], var[:], musq[:], ALU.subtract)
                nc.vector.tensor_scalar(var[:], var[:], EPS, None, ALU.add)
                nc.scalar.sqrt(var[:], var[:])
                nc.vector.reciprocal(var[:], var[:])          # 1/sigma
                nc.vector.tensor_mul(gp[:], vcol(gname), var[:])
                nc.vector.tensor_mul(bp[:], gp[:], mu[:])
                nc.vector.tensor_tensor(bp[:], vcol(bname), bp[:], ALU.subtract)
                return gp, bp

            def fold_bn(wsrc, gp, bp, bias_name, tag):
                """Fold BN (gp, bp) into next layer: W' = diag(gp)W bf16,
                bias' = bp@W + b_next [128, 3] fp32."""
                wp = const.tile([128, 3, D], bf16, name=f"Wf_{tag}", tag=f"Wf_{tag}")
                for k in range(3):
                    nc.vector.tensor_scalar(
                        wp[:, k, :], wsrc[:, k, :], gp[:, k:k + 1], None, ALU.mult)
                bpb = spool.tile([128, 3], bf16, tag=tag + "_bpb")
                nc.vector.tensor_copy(bpb[:], bp[:])
                ps_b = typs.tile([128, GPC], fp32, tag="tiny")
                for m in range(3):
                    for k in range(3):
                        nc.tensor.matmul(
                            ps_b[:, m:m + 1],
                            lhsT=wsrc[:, k, m * 128:(m + 1) * 128],
                            rhs=bpb[:, k:k + 1], start=(k == 0), stop=(k == 2))
                bias = spool.tile([128, 3], fp32, tag=tag + "_bi")
                nc.vector.tensor_tensor(
                    bias[:], ps_b[:, 0:3], vcol(bias_name), ALU.add)
                return wp, bias

            def mlp_layer(x_t, w_t, bias_ap, tag, bias_is_tile=False):
                """x_t [128,3,NPC] bf16 -> relu(x@W + b) bf16 [128,3,NPC]
                + [128, 6] sum/sumsq stats via activation accum."""
                out = big.tile([128, 3, NPC], bf16, tag="big")
                sums = spool.tile([128, 3, NCH], fp32, tag=tag + "_sc")
                sqs = spool.tile([128, 3, NCH], fp32, tag=tag + "_qc")
                # j-outer so output COLUMNS complete early: the y-build
                # transposes (and other column consumers) can then overlap
                # this layer's later chunks instead of waiting for m=2.
                for j in range(NCH):
                    for m in range(3):
                        bcol = bias_ap[:, m:m + 1]
                        sl = slice(j * NCOL, (j + 1) * NCOL)
                        ps = mmps.tile([128, NCOL], fp32, tag="mm")
                        for k in range(3):
                            nc.tensor.matmul(
                                ps[:], lhsT=w_t[:, k, m * 128:(m + 1) * 128],
                                rhs=x_t[:, k, sl], start=(k == 0), stop=(k == 2))
                        nc.scalar.activation(out[:, m, sl], ps[:], AF.Relu,
                                             bias=bcol,
                                             accum_out=sums[:, m, j:j + 1])
                        sq = sqpool.tile([128, NCOL], fp32, name="sqscr", tag="sqscr")
                        if m == 0:   # balance sumsq work across Act and DVE
                            nc.scalar.activation(sq[:], out[:, m, sl], AF.Square,
                                                 accum_out=sqs[:, m, j:j + 1])
                        else:
                            nc.vector.tensor_mul(sq[:], out[:, m, sl], out[:, m, sl])
                            nc.vector.reduce_sum(sqs[:, m, j:j + 1], sq[:], axis=AX.X)
                st = spool.tile([128, 6], fp32, tag=tag + "_st")
                for m in range(3):
                    nc.vector.reduce_sum(st[:, 2 * m:2 * m + 1], sums[:, m, :], axis=AX.X)
                    nc.vector.reduce_sum(st[:, 2 * m + 1:2 * m + 2], sqs[:, m, :], axis=AX.X)
                return out, st

            # ---------------- pre-MLPs (xT_t loaded in the const section)
            x1, st1 = mlp_layer(xT_t, W["w1"], vcol("b1"), "l1")
            red1 = gather_stats(st1[:], 6, "ar1")
            gp1, bp1 = bn_coeffs(red1, 3, N, "g0", "be0", "bn1")
            w2f, bias2 = fold_bn(W["w2"], gp1, bp1, "b2", "f2")
            tap("x1", x1[:])

            x2, st2 = mlp_layer(x1, w2f, bias2, "l2", bias_is_tile=True)
            # st2 is stats of RAW r2 (pre-BN2) — exactly what BN2 needs.
            tap("x2", x2[:])
            if stage <= 2:
                red2d = gather_stats(st2[:], 6, "ar2")
                finish_early(red2d[0:C, 0:6], width=6)
                continue

            # ---------------- y = r2 * dinv (node-major bf16) ; AllGather
            # ybuf kept in SBUF for the self-loop PSUM seeds.
            ybuf = const.tile([128, NW, D], bf16)
            y_slice = dram.tile([NPC, D], bf16, tag="y_slice")
            for w in range(NW):
                for m in range(3):
                    trp = trps.tile([128, 128], bf16, tag="tr")
                    nc.tensor.transpose(
                        trp[:], x2[:, m, w * 128:(w + 1) * 128], ident[:])
                    if m == 1:   # split the dinv scaling across Act and DVE
                        nc.scalar.activation(
                            ybuf[:, w, m * 128:(m + 1) * 128], trp[:],
                            AF.Copy, scale=dinv[:, w:w + 1])
                    else:
                        nc.vector.tensor_scalar(
                            ybuf[:, w, m * 128:(m + 1) * 128], trp[:],
                            dinv[:, w:w + 1], None, ALU.mult)
                nc.sync.dma_start(y_slice[w * 128:(w + 1) * 128, :], ybuf[:, w, :])

            y_full = dram.tile([N, D], bf16, tag="y_full", addr_space="Shared")
            nc.gpsimd.collective_compute(
                "AllGather", ALU.bypass, replica_groups=grp,
                ins=[y_slice[:]], outs=[y_full[:]])
            load_deferred(ybuf[0:1, NW - 1, 0:1])

            # BN2 stats sync + folds overlap with the AllGather / gathers.
            red2 = gather_stats(st2[:], 6, "ar2")
            gp2, bp2 = bn_coeffs(red2, 3, N, "g1", "be1", "bn2")
            # only the weight scaling is folded for the conv: the bp2@Wc term
            # enters scaled per-node by u_d (tWc path below), not uniformly.
            wcf = const.tile([128, 3, D], bf16, name="Wf_fc", tag="Wf_fc")
            for k in range(3):
                nc.vector.tensor_scalar(
                    wcf[:, k, :], W["wc"][:, k, :], gp2[:, k:k + 1], None, ALU.mult)
            biasc = vcol("bcv")
            # tWc row [3, 128] fp32: (bp2 @ Wc) per m-chunk, transposed
            bpb2 = spool.tile([128, 3], bf16, tag="bpb2")
            nc.vector.tensor_copy(bpb2[:], bp2[:])
            ps_t = typs.tile([128, GPC], fp32, tag="tiny")
            for m in range(3):
                for k in range(3):
                    nc.tensor.matmul(
                        ps_t[:, m:m + 1],
                        lhsT=W["wc"][:, k, m * 128:(m + 1) * 128],
                        rhs=bpb2[:, k:k + 1], start=(k == 0), stop=(k == 2))
            twc_col = spool.tile([128, 3], fp32, tag="twc_c")
            nc.vector.tensor_copy(twc_col[:], ps_t[:, 0:3])
            twcT = spool.tile([1, 3 * 128], fp32, tag="twcT")
            for m in range(3):
                # per-column transpose so every PSUM read starts at partition 0
                # (walrus rejects partition-offset reads)
                trp_t = tfps.tile([128, 128], fp32, tag="trf")
                nc.tensor.transpose(trp_t[0:1, :], twc_col[:, m:m + 1], identf[:])
                nc.vector.tensor_copy(twcT[:, m * 128:(m + 1) * 128],
                                      trp_t[0:1, :])
            if stage <= 3:
                yck = spool.tile([128, D], bf16, name="yck", tag="yck")
                nc.sync.dma_start(yck[:], y_full[0:128, :])
                finish_early(yck[0:C, 0:GPC])
                continue

            # ---------------- sentence branch (emitted here to hide under
            # the AllGather): H_sentT [128, 6, GPC]
            HsT = spool.tile([128, 6, GPC], fp32, tag="HsT")
            for b in range(GPC):
                ps_ht = typs.tile([128, GPC], fp32, tag="tiny")
                hts = []
                for hsrc in (I["lasth"], I["firsth"]):
                    for sc in range(4):
                        ht = hspool.tile([128, E], bf16, name="ht", tag="ht")
                        if b == 0:
                            # gate behind y-build so these transfers hide
                            # under the AllGather instead of delaying L1
                            nc.vector.tensor_copy(ht[0:1, 0:1],
                                                  ybuf[0:1, NW - 1, 0:1])
                        nc.sync.dma_start(
                            ht[:], hsrc[b * S + sc * 128:b * S + (sc + 1) * 128, :])
                        hts.append(ht)
                for m in range(6):
                    for i, ht in enumerate(hts):
                        nc.tensor.matmul(
                            ps_ht[:, m:m + 1],
                            lhsT=ht[:, m * 128:(m + 1) * 128],
                            rhs=onescol[:],
                            start=(i == 0), stop=(i == 7))
                nc.vector.tensor_scalar(
                    HsT[:, :, b], ps_ht[:, 0:6],
                    1.0 / (2 * S), None, ALU.mult)
            tap("hsT", HsT[:])

            # ---------------- conv: per dst window, gather + selector matmul
            # into node-major PSUM A; then Wc' after aggregation.
            convT = big.tile([128, 3, NPC], bf16, tag="big")
            csums = spool.tile([128, 3, NW], fp32, tag="cv_sc")
            csqs = spool.tile([128, 3, NW], fp32, tag="cv_qc")
            for w in range(NW):
                ps_c = cvps.tile([128, D], fp32, tag="cv")
                # self-loop seed: A += I @ ybuf[w]
                nc.tensor.matmul(ps_c[:], lhsT=ident[:], rhs=ybuf[:, w, :],
                                 start=True, stop=False)
                nt = tpw[w]
                t0 = tstart[w]
                # balanced chunk sizes: a trailing 1-tile gather stalls the
                # DMA pipeline (desc-gen 1.3us > its own transfer time)
                nchunks = cdiv(nt, GCHUNK)
                base, extra = divmod(nt, nchunks)
                csizes = [base + (1 if i < extra else 0) for i in range(nchunks)]
                done = 0
                for cn in csizes:
                    gt = gpool.tile([128, GCHUNK * D], bf16, tag="g")
                    nc.gpsimd.dma_gather(
                        out_ap=gt[:, :cn * D].rearrange("p (t f) -> p t f", f=D),
                        in_ap=y_full[:],
                        idxs_ap=idx_t[:, (t0 + done) * 8:(t0 + done + cn) * 8],
                        num_idxs=cn * 128, num_idxs_reg=cn * 128, elem_size=D)
                    gv = gt[:, :cn * D].rearrange("p (t f) -> p t f", f=D)
                    for tl in range(cn):
                        tg = t0 + done + tl
                        sel = selp.tile([128, 128], bf16, tag="sel")
                        nc.vector.tensor_scalar(sel[:], iota[:], dst_t[:, tg:tg + 1],
                                                None, ALU.is_equal)
                        last = (done + tl == nt - 1)
                        nc.tensor.matmul(ps_c[:], lhsT=sel[:], rhs=gv[:, tl, :],
                                         start=False, stop=last)
                    done += cn
                # aggS = A * dinv[dst] (scale per partition) on Act
                aggS = spool.tile([128, D], bf16, tag="aggS")
                nc.scalar.activation(aggS[:], ps_c[:], AF.Copy,
                                     scale=dinv[:, w:w + 1])
                # transpose to feature-major, then Wc' + u*tWc + bias, relu
                aggT = spool.tile([128, 3, 128], bf16, tag="aggT")
                for m in range(3):
                    trp = trps.tile([128, 128], bf16, tag="tr")
                    nc.tensor.transpose(trp[:], aggS[:, m * 128:(m + 1) * 128],
                                        ident[:])
                    nc.scalar.activation(aggT[:, m, :], trp[:], AF.Copy)
                for m in range(3):
                    ps2f = mmps.tile([128, NCOL], fp32, tag="mm")
                    ps2 = ps2f[:, 0:128]
                    for k in range(3):
                        nc.tensor.matmul(
                            ps2[:], lhsT=wcf[:, k, m * 128:(m + 1) * 128],
                            rhs=aggT[:, k, :], start=(k == 0), stop=False)
                    nc.tensor.matmul(
                        ps2[:], lhsT=twcT[:, m * 128:(m + 1) * 128],
                        rhs=u_row[:, w * 128:(w + 1) * 128],
                        start=False, stop=True)
                    nc.scalar.activation(convT[:, m, w * 128:(w + 1) * 128],
                                         ps2[:], AF.Relu,
                                         bias=biasc[:, m:m + 1],
                                         accum_out=csums[:, m, w:w + 1])
                    sq = sqpool.tile([128, NCOL], fp32, name="sqscr2", tag="sqscr")
                    nc.vector.tensor_mul(sq[:, 0:128],
                                         convT[:, m, w * 128:(w + 1) * 128],
                                         convT[:, m, w * 128:(w + 1) * 128])
                    nc.vector.reduce_sum(csqs[:, m, w:w + 1], sq[:, 0:128], axis=AX.X)
            if stage <= 3.9:
                finish_early(convT[0:C, 0, 0:GPC])
                continue
            st4 = spool.tile([128, 6], fp32, tag="st4")
            for m in range(3):
                nc.vector.reduce_sum(st4[:, 2 * m:2 * m + 1], csums[:, m, :], axis=AX.X)
                nc.vector.reduce_sum(st4[:, 2 * m + 1:2 * m + 2], csqs[:, m, :], axis=AX.X)
            tap("convT", convT[:])
            red4 = gather_stats(st4[:], 6, "ar4")
            gp4, bp4 = bn_coeffs(red4, 3, N, "g4", "be4", "bn4")
            wp1f, biasp1 = fold_bn(W["wp1"], gp4, bp4, "bp1", "f4")
            if stage <= 4:
                finish_early(convT[0:C, 0, 0:GPC])
                continue

            # ---------------- post MLPs
            p1, st5 = mlp_layer(convT, wp1f, biasp1, "l5", bias_is_tile=True)
            red5 = gather_stats(st5[:], 6, "ar5")
            gp5, bp5 = bn_coeffs(red5, 3, N, "g5", "be5", "bn5")
            wp2f, biasp2 = fold_bn(W["wp2"], gp5, bp5, "bp2", "f5")

            # post2: row-major bf16 to DRAM (pre-BN6); stats via accum
            p2_dram = dram.tile([NPC, D], bf16, tag="p2")
            p2sc_s = spool.tile([128, 3, NCH], fp32, tag="p2s")
            p2sc_q = spool.tile([128, 3, NCH], fp32, tag="p2q")
            for j in range(NCH):
                sl = slice(j * NCOL, (j + 1) * NCOL)
                p2c = spool.tile([128, 3, NCOL], bf16, tag="p2c")
                for m in range(3):
                    ps = mmps.tile([128, NCOL], fp32, tag="mm")
                    for k in range(3):
                        nc.tensor.matmul(
                            ps[:], lhsT=wp2f[:, k, m * 128:(m + 1) * 128],
                            rhs=p1[:, k, sl], start=(k == 0), stop=(k == 2))
                    nc.scalar.activation(p2c[:, m, :], ps[:], AF.Relu,
                                         bias=biasp2[:, m:m + 1],
                                         accum_out=p2sc_s[:, m, j:j + 1])
                    sq = sqpool.tile([128, NCOL], fp32, name="sqscr3", tag="sqscr")
                    if m == 0:
                        nc.scalar.activation(sq[:], p2c[:, m, :], AF.Square,
                                             accum_out=p2sc_q[:, m, j:j + 1])
                    else:
                        nc.vector.tensor_mul(sq[:], p2c[:, m, :], p2c[:, m, :])
                        nc.vector.reduce_sum(p2sc_q[:, m, j:j + 1], sq[:], axis=AX.X)
                for nb in range(NCOL // 128):
                    rmw = spool.tile([128, D], bf16, tag="rmw")
                    for m in range(3):
                        trp = trps.tile([128, 128], bf16, tag="tr")
                        nc.tensor.transpose(
                            trp[:], p2c[:, m, nb * 128:(nb + 1) * 128], ident[:])
                        if m == 1:   # split PSUM->SBUF copies across engines
                            nc.scalar.activation(rmw[:, m * 128:(m + 1) * 128],
                                                 trp[:], AF.Copy)
                        else:
                            nc.vector.tensor_copy(rmw[:, m * 128:(m + 1) * 128],
                                                  trp[:])
                    nc.sync.dma_start(
                        p2_dram[j * NCOL + nb * 128:j * NCOL + (nb + 1) * 128, :],
                        rmw[:])
            st6 = spool.tile([128, 6], fp32, tag="st6")
            for m in range(3):
                nc.vector.reduce_sum(st6[:, 2 * m:2 * m + 1], p2sc_s[:, m, :], axis=AX.X)
                nc.vector.reduce_sum(st6[:, 2 * m + 1:2 * m + 2], p2sc_q[:, m, :], axis=AX.X)
            red6 = gather_stats(st6[:], 6, "ar6")
            gp6, bp6 = bn_coeffs(red6, 3, N, "g6", "be6", "bn6")
            if stage <= 5:
                finish_early(red6[0:C, 0:6], width=6)
                continue

            # ---------------- masked-node gather -> flT [128, 3, 16] bf16 (BN6'd)
            gth = spool.tile([128, D], bf16, tag="gth")
            nc.gpsimd.dma_gather(
                out_ap=gth[:].rearrange("p (t f) -> p t f", f=D),
                in_ap=p2_dram[:], idxs_ap=gidx_t[:],
                num_idxs=16, num_idxs_reg=16, elem_size=D)
            flT = spool.tile([128, 3, 16], bf16, tag="flT")
            for m in range(3):
                trp_full = trps.tile([128, 128], bf16, tag="tr")
                trp = trp_full[:, 0:16]
                nc.tensor.matmul(trp, lhsT=gth[0:16, m * 128:(m + 1) * 128],
                                 rhs=ident[0:16, 0:16], is_transpose=True)
                nc.vector.tensor_scalar(flT[:, m, :], trp,
                                        gp6[:, m:m + 1], bp6[:, m:m + 1],
                                        ALU.mult, ALU.add)
            tap("flT", flT[:])

            # ---------------- tail: outc, H_sent BN, att, out
            outcT = spool.tile([128, 6, GPC], fp32, tag="outcT")
            for m in range(6):
                ps_o = typs.tile([128, GPC], fp32, tag="tiny")
                for k in range(6):
                    kc, kj = k % 3, k // 3
                    nc.tensor.matmul(
                        ps_o[:], lhsT=wcat[:, k, m * 128:(m + 1) * 128],
                        rhs=flT[:, kc, kj::2], start=(k == 0), stop=(k == 5))
                nc.scalar.activation(outcT[:, m, :], ps_o[:], AF.Relu,
                                     bias=vcol("bcat")[:, m:m + 1])
            stt = spool.tile([128, 24], fp32, tag="stt")
            for m in range(6):
                nc.vector.reduce_sum(stt[:, 2 * m:2 * m + 1], outcT[:, m, :], axis=AX.X)
                sq = spool.tile([128, GPC], fp32, tag="ttsq")
                nc.scalar.square(sq[:], outcT[:, m, :])
                nc.vector.reduce_sum(stt[:, 2 * m + 1:2 * m + 2], sq[:], axis=AX.X)
                nc.vector.reduce_sum(stt[:, 12 + 2 * m:13 + 2 * m], HsT[:, m, :], axis=AX.X)
                nc.scalar.square(sq[:], HsT[:, m, :])
                nc.vector.reduce_sum(stt[:, 13 + 2 * m:14 + 2 * m], sq[:], axis=AX.X)
            redt = gather_stats(stt[:], 24, "art")
            gpc_, bpc_ = bn_coeffs(redt[:, 0:12], 6, B, "gc0", "bc0", "bnc")
            gph, bph = bn_coeffs(redt[:, 12:24], 6, B, "gc1", "bc1", "bnh")
            attT = spool.tile([128, 6, GPC], fp32, tag="attT")
            for m in range(6):
                nc.vector.tensor_scalar(attT[:, m, :], HsT[:, m, :],
                                        gph[:, m:m + 1], bph[:, m:m + 1],
                                        ALU.mult, ALU.add)
                nc.vector.tensor_scalar(outcT[:, m, :], outcT[:, m, :],
                                        gpc_[:, m:m + 1], bpc_[:, m:m + 1],
                                        ALU.mult, ALU.add)
                nc.vector.tensor_add(attT[:, m, :], attT[:, m, :], outcT[:, m, :])
            ps_ft = typs.tile([128, GPC], fp32, tag="tiny")
            ps_f = ps_ft[0:C, :]
            for k in range(6):
                nc.tensor.matmul(ps_f, lhsT=wout[:, k, :], rhs=attT[:, k, :],
                                 start=(k == 0), stop=False)
            nc.tensor.matmul(ps_f, lhsT=brow[0:1, D:D + C], rhs=ones8[:],
                             start=False, stop=True)
            fin = spool.tile([C, GPC], fp32, tag="fin")
            nc.vector.tensor_copy(fin[:], ps_f)
            nc.sync.dma_start(outT[:], fin[:])

    nc.compile()
    return nc, tap_outs


# ---------------------------------------------------------------- entry
_CACHE = {}


def _get_compiled(meta):
    key = meta
    if key not in _CACHE:
        nc, _ = build(meta)
        split_waits(nc)
        _CACHE[key] = nc
    return _CACHE[key]


def kernel(**inputs):
    in_maps, meta = preprocess(inputs)
    nc = _get_compiled(meta)
    from concourse import bass2jax
    results = bass2jax.run_bass_via_pjrt(nc, in_maps, n_cores=NCORES)
    out = np.concatenate([results[c]["outT"].T for c in range(NCORES)], axis=0)
    return out.astype(np.float32)


# revision 48
# speedup vs baseline: 1.0097x; 1.0097x over previous
"""Trainium2 Bass kernel for nn_BaselineModelWithGNN (8-core SPMD).

Self-contained: hardcodes shapes/sharding; builds, compiles and runs the Bass
program on 8 NeuronCores via the axon PJRT path.

Key observation: the reference applies each of the 3 GCN convs to the same
input x and overwrites `out`, so only conv i=2 (w_conv[2], b_conv[2],
bng[4]) affects the result — one conv is computed.

Sharding: nodes (and their incident edges, dst-sharded) are partitioned
contiguously across the 8 cores (4096 nodes / 8 graphs per core); the
PLM/pooling branch is data-parallel over batch.

Optimizations vs the naive mapping:
- BatchNorm folding: BN is affine given its batch stats, so each BN is folded
  into the NEXT linear layer's weights/bias on device (W' = diag(g/s)W,
  b' = t@W + b_next). No full-tensor BN application passes.
- The GCN aggregates RAW y = dinv*relu(z2) (pre-BN2); the BN2 affine
  correction commutes through the (linear) aggregation:
    conv_pre[d] = dinv_d*(A[d] @ Wc') + u_d*(t2@Wc) + b_conv
  with A[d] = sum_src y[src], u_d = dinv_d*S1[d], S1[d] = sum_src dinv[src]
  (host-precomputed from degrees). So the edge-gather phase does not wait on
  the BN2 stats sync.
- W_conv is applied AFTER aggregation (they commute), removing the z matmul
  from the pre-collective critical path.
- Self-loop contributions are seeded into PSUM via an identity matmul
  instead of gathered (self-loops dropped from the edge list).
- BN batch stats are synced with AllGather + local sum (cheaper than
  AllReduce in the profile model); stat sums come free from the Activation
  engine's accum_out during the relu pass.
- The sentence branch is emitted after the y AllGather so its DMA/PE work
  hides under the collective.
"""
import sys
sys.path.insert(0, "/opt/trn_rl_repo")
from contextlib import ExitStack

import numpy as np
import ml_dtypes

import bass_rust as _br
import concourse.bacc as bacc
import concourse.bass as bass
import concourse.tile as tile
from concourse import mybir
from concourse._compat import cdiv

fp32 = mybir.dt.float32
bf16 = mybir.dt.float16  # fp16 for 8x less quant noise than bf16
i16 = mybir.dt.int16
AF = mybir.ActivationFunctionType
ALU = mybir.AluOpType
AX = mybir.AxisListType

NCORES = 8
B, S, E = 64, 512, 768
D = 384
NG = 512
N = B * NG              # 32768
NEDGE = 1048576
C = 3
NPC = N // NCORES       # 4096 nodes per core
GPC = B // NCORES       # 8 graphs per core
NW = NPC // 128         # 32 dst windows per core
NCOL = 512
NCH = NPC // NCOL       # 8 column chunks
EPS = 1e-5
GCHUNK = 12             # gather chunk: tiles (of 128 edges) per dma_gather


# ---------------------------------------------------------------- BIR patch
def split_waits(nc):
    """walrus here supports ONE sync-wait per instruction; split extras onto
    NoOps inserted just before, on the same engine."""
    counter = 0
    for f in nc.m.functions:
        for bb in f.blocks:
            newlist, changed = [], False
            for inst in bb.instructions:
                si = inst.sync_info
                if si is not None and len(si.on_wait) > 1:
                    waits = list(si.on_wait)
                    for w in waits[:-1]:
                        counter += 1
                        nop = mybir.InstNoOp(name=f"I-WSPLIT-{counter}", ins=[], outs=[])
                        nop.engine = inst.engine
                        nop.sync_info = _br.SyncInfo(on_wait=[w], on_update=[])
                        newlist.append(nop)
                    inst.sync_info = _br.SyncInfo(
                        on_wait=[waits[-1]], on_update=list(si.on_update))
                    changed = True
                newlist.append(inst)
            if changed:
                bb.instructions = newlist


# ---------------------------------------------------------------- host prep
def _col3(v):
    """[384] -> [128, 3] column layout (feature f = c*128+p)."""
    return np.ascontiguousarray(np.asarray(v).reshape(3, 128).T).astype(np.float32)


def _col6(v):
    return np.ascontiguousarray(np.asarray(v).reshape(6, 128).T).astype(np.float32)


def _wchunks(w, kc, m):
    """[K, M] -> [128, kc, M] (k-chunk on partitions)."""
    K, M = w.shape
    assert K == kc * 128
    return np.ascontiguousarray(w.reshape(kc, 128, M).transpose(1, 0, 2))


def _wrap_idx(idx):
    """int16 idx array (len % 128 == 0) -> [128, len/16] dma_gather layout."""
    blk = idx.reshape(-1, 16).T  # [16, len/16]
    return np.ascontiguousarray(np.tile(blk, (8, 1)))


def preprocess(inputs):
    ei = np.asarray(inputs["edge_index"]).astype(np.int64)
    loop = np.arange(N, dtype=np.int64)
    dst_all = np.concatenate([ei[1], loop])

    deg = np.bincount(dst_all, minlength=N).astype(np.float32)
    dinv = 1.0 / np.sqrt(deg)
    # S1[d] = sum over in-edges (incl self-loop) of dinv[src]; u = dinv*S1
    S1 = np.bincount(ei[1], weights=dinv[ei[0]], minlength=N) + dinv
    u = (dinv * S1).astype(np.float32)

    # self-loops handled on device (PSUM identity seed): edges only
    src_e = ei[0]
    dst_e = ei[1]
    win = dst_e >> 7
    order = np.lexsort((src_e, win))
    src_s = src_e[order]
    dst_s = dst_e[order]

    wcnt = np.bincount(win, minlength=N // 128)          # [256]
    tpw = np.maximum(
        np.ceil(wcnt.reshape(NCORES, NW) / 128).max(axis=0), 1
    ).astype(np.int64)                                    # [32] shared schedule
    T_total = int(tpw.sum())
    wstart = np.zeros(N // 128 + 1, np.int64)
    np.cumsum(wcnt, out=wstart[1:])
    tstart = np.zeros(NW + 1, np.int64)
    np.cumsum(tpw, out=tstart[1:])

    # masked node indices (2 per graph, ascending)
    mask = np.asarray(inputs["graph_masking"])
    sel = np.argsort(-mask, axis=1, kind="stable")[:, :2]  # top_k: ones, asc idx
    sel = np.sort(sel, axis=1)

    xT = np.zeros((D, N), np.float32)
    xT[:300] = np.asarray(inputs["x_nodes"]).T
    xT = xT.astype(np.float16)

    w1p = np.zeros((D, D), np.float32)
    w1p[:300] = np.asarray(inputs["w_pre1"])

    lastf = np.asarray(inputs["last_h"]).astype(np.float16)
    firstf = np.asarray(inputs["first_h"]).astype(np.float16)

    bng_g, bng_b = np.asarray(inputs["bng_g"]), np.asarray(inputs["bng_b"])
    bn_g, bn_b = np.asarray(inputs["bn_g"]), np.asarray(inputs["bn_b"])
    # vec columns [128, 75]: order documented here, mirrored on device
    cols = [
        _col3(inputs["b_pre1"]), _col3(inputs["b_pre2"]),
        _col3(inputs["b_post1"]), _col3(inputs["b_post2"]),
        _col3(np.asarray(inputs["b_conv"])[2]),
        _col6(inputs["b_cat"]),
        _col3(bng_g[0]), _col3(bng_b[0]), _col3(bng_g[1]), _col3(bng_b[1]),
        _col3(bng_g[4]), _col3(bng_b[4]), _col3(bng_g[5]), _col3(bng_b[5]),
        _col3(bng_g[6]), _col3(bng_b[6]),
        _col6(bn_g[0]), _col6(bn_b[0]), _col6(bn_g[1]), _col6(bn_b[1]),
    ]
    vecs = np.concatenate(cols, axis=1)  # [128, 3*5+6+3*10+6*4] = [128, 75]
    brow = np.zeros((1, 512), np.float32)
    brow[0, D:D + C] = np.asarray(inputs["b_out"])

    w_bf = {
        "w1": _wchunks(w1p, 3, D).astype(np.float16),
        "w2": _wchunks(np.asarray(inputs["w_pre2"]), 3, D).astype(np.float16),
        "wc": _wchunks(np.asarray(inputs["w_conv"])[2], 3, D).astype(np.float16),
        "wp1": _wchunks(np.asarray(inputs["w_post1"]), 3, D).astype(np.float16),
        "wp2": _wchunks(np.asarray(inputs["w_post2"]), 3, D).astype(np.float16),
    }
    wcat = _wchunks(np.asarray(inputs["w_cat"]), 6, E).astype(np.float16)
    wout = _wchunks(np.asarray(inputs["w_out"]), 6, C).astype(np.float32)

    in_maps = []
    for c in range(NCORES):
        n0 = c * NPC
        src_pad = np.zeros(T_total * 128, np.int64)
        dstloc = np.full(T_total * 128, -1.0, np.float32)
        for w in range(NW):
            gw = c * NW + w
            a, b_ = wstart[gw], wstart[gw + 1]
            k = b_ - a
            pos = tstart[w] * 128
            src_pad[pos:pos + k] = src_s[a:b_]
            dstloc[pos:pos + k] = (dst_s[a:b_] - gw * 128).astype(np.float32)
        # int16: N-1 = 32767 fits exactly
        idx_w = _wrap_idx(src_pad.astype(np.int16))            # [128, T*8]
        dst_t = np.ascontiguousarray(dstloc.reshape(T_total, 128).T)  # [128, T]

        deg_nm = np.ascontiguousarray(
            deg[n0:n0 + NPC].reshape(NW, 128).T)               # [128, 32]
        u_row = u[n0:n0 + NPC].reshape(1, NPC)                 # [1, 4096]

        gidx = (sel[c * GPC:(c + 1) * GPC] +
                np.arange(c * GPC, (c + 1) * GPC)[:, None] * NG - n0)  # local
        gidx = gidx.reshape(-1).astype(np.int16)               # [16]
        gidx_w = np.zeros((128, 1), np.int16)
        gidx_w[:16, 0] = gidx
        gidx_w = np.tile(gidx_w[:16], (8, 1))

        m = {
            "lasth": np.ascontiguousarray(
                lastf[c * GPC:(c + 1) * GPC].reshape(GPC * S, E)),
            "firsth": np.ascontiguousarray(
                firstf[c * GPC:(c + 1) * GPC].reshape(GPC * S, E)),
            "xT": np.ascontiguousarray(
                xT.reshape(3, 128, N)[:, :, n0:n0 + NPC].transpose(1, 0, 2)
            ).reshape(128, 3 * NPC),
            "eidx": idx_w, "dstloc": dst_t, "deg": deg_nm, "urow": u_row,
            "vecs": vecs, "brow": brow, "gidx": gidx_w,
            "w1": w_bf["w1"].reshape(128, 3 * D),
            "w2": w_bf["w2"].reshape(128, 3 * D),
            "wc": w_bf["wc"].reshape(128, 3 * D),
            "wp1": w_bf["wp1"].reshape(128, 3 * D),
            "wp2": w_bf["wp2"].reshape(128, 3 * D),
            "wcat": wcat.reshape(128, 6 * E),
            "wout": wout.reshape(128, 6 * C),
        }
        in_maps.append(m)
    meta = (tuple(int(t) for t in tpw),)
    return in_maps, meta


# ---------------------------------------------------------------- device
def build(meta, rep=1, taps=(), stage=99):
    tpw = meta[0]
    T_total = sum(tpw)
    tstart = [0]
    for t in tpw:
        tstart.append(tstart[-1] + t)

    nc = bacc.Bacc("TRN2", dynamic_dma_scratch_size=65536)
    I = {}
    I["lasth"] = nc.dram_tensor("lasth", [GPC * S, E], bf16, kind="ExternalInput")
    I["firsth"] = nc.dram_tensor("firsth", [GPC * S, E], bf16, kind="ExternalInput")
    I["xT"] = nc.dram_tensor("xT", [128, 3 * NPC], bf16, kind="ExternalInput")
    I["eidx"] = nc.dram_tensor("eidx", [128, T_total * 8], i16, kind="ExternalInput")
    I["dstloc"] = nc.dram_tensor("dstloc", [128, T_total], fp32, kind="ExternalInput")
    I["deg"] = nc.dram_tensor("deg", [128, NW], fp32, kind="ExternalInput")
    I["urow"] = nc.dram_tensor("urow", [1, NPC], fp32, kind="ExternalInput")
    I["vecs"] = nc.dram_tensor("vecs", [128, 75], fp32, kind="ExternalInput")
    I["brow"] = nc.dram_tensor("brow", [1, 512], fp32, kind="ExternalInput")
    I["gidx"] = nc.dram_tensor("gidx", [128, 1], i16, kind="ExternalInput")
    for w in ("w1", "w2", "wc", "wp1", "wp2"):
        I[w] = nc.dram_tensor(w, [128, 3 * D], bf16, kind="ExternalInput")
    I["wcat"] = nc.dram_tensor("wcat", [128, 6 * E], bf16, kind="ExternalInput")
    I["wout"] = nc.dram_tensor("wout", [128, 6 * C], fp32, kind="ExternalInput")
    outT = nc.dram_tensor("outT", [C, GPC], fp32, kind="ExternalOutput")
    tap_outs = {}

    grp = [list(range(NCORES))]

    with tile.TileContext(nc) as tc, ExitStack() as ctx:
        const = ctx.enter_context(tc.tile_pool(name="const", bufs=1))
        big = ctx.enter_context(tc.tile_pool(name="big", bufs=2))
        gpool = ctx.enter_context(tc.tile_pool(name="gath", bufs=3))
        spool = ctx.enter_context(tc.tile_pool(name="small", bufs=2))
        selp = ctx.enter_context(tc.tile_pool(name="sel", bufs=8))
        hspool = ctx.enter_context(tc.tile_pool(name="hs", bufs=8))
        sqpool = ctx.enter_context(tc.tile_pool(name="sq", bufs=2))
        mmps = ctx.enter_context(tc.tile_pool(name="mmps", bufs=2, space="PSUM"))
        cvps = ctx.enter_context(tc.tile_pool(name="cvps", bufs=2, space="PSUM"))
        trps = ctx.enter_context(tc.tile_pool(name="trps", bufs=2, space="PSUM"))
        typs = ctx.enter_context(tc.tile_pool(name="typs", bufs=1, space="PSUM"))
        tfps = ctx.enter_context(tc.tile_pool(name="tfps", bufs=1, space="PSUM"))
        dram = ctx.enter_context(tc.tile_pool(name="dram", bufs=1, space="DRAM"))

        # ---------------- constants
        iota = const.tile([128, 128], fp32)
        nc.gpsimd.iota(iota[:], pattern=[[1, 128]], base=0, channel_multiplier=0,
                       allow_small_or_imprecise_dtypes=True)
        pidx = const.tile([128, 1], fp32)  # partition index column
        nc.gpsimd.iota(pidx[:], pattern=[[0, 1]], base=0, channel_multiplier=1,
                       allow_small_or_imprecise_dtypes=True)
        ident = const.tile([128, 128], bf16)
        nc.vector.tensor_scalar(ident[:], iota[:], pidx[:], None, ALU.is_equal)
        identf = const.tile([128, 128], fp32)
        nc.vector.tensor_scalar(identf[:], iota[:], pidx[:], None, ALU.is_equal)

        # head-critical loads only; gather-phase data (idx/dst/u/gidx) and
        # tail data (wcat/wout/brow) are DMA'd after the AllGather is issued.
        # xT first: it gates L1, everything else here is small.
        xT_t = big.tile([128, 3, NPC], bf16, tag="big")
        nc.sync.dma_start(xT_t[:], I["xT"][:].rearrange("p (k n) -> p k n", k=3))
        deg_t = const.tile([128, NW], fp32)
        nc.sync.dma_start(deg_t[:], I["deg"][:])
        vecs = const.tile([128, 75], fp32)
        nc.sync.dma_start(vecs[:], I["vecs"][:])
        W = {}
        for w in ("w1", "w2", "wc", "wp1", "wp2"):
            W[w] = const.tile([128, 3, D], bf16, name=f"W_{w}", tag=f"W_{w}")
            nc.sync.dma_start(W[w][:], I[w][:].rearrange("p (k m) -> p k m", k=3))
        idx_t = const.tile([128, T_total * 8], i16)
        dst_t = const.tile([128, T_total], fp32)
        u_row = const.tile([1, NPC], fp32)
        gidx_t = const.tile([128, 1], i16)
        wcat = const.tile([128, 6, E], bf16)
        wout = const.tile([128, 6, C], fp32)
        brow = const.tile([1, 512], fp32)

        def load_deferred(gate_ap):
            """Gate the deferred loads behind `gate_ap` (a WAW dummy write into
            each destination) so the tile scheduler cannot front-load their
            DMA transfers into the head where they'd steal DMA bandwidth."""
            gate32 = const.tile([1, 1], fp32)
            nc.vector.tensor_copy(gate32[:], gate_ap)
            for ap in (idx_t[0:1, 0:1], dst_t[0:1, 0:1], u_row[0:1, 0:1],
                       gidx_t[0:1, 0:1], wcat[0:1, 0, 0:1], wout[0:1, 0, 0:1],
                       brow[0:1, 0:1]):
                nc.vector.tensor_copy(ap, gate32[:])
            nc.sync.dma_start(idx_t[:], I["eidx"][:])
            nc.sync.dma_start(dst_t[:], I["dstloc"][:])
            nc.sync.dma_start(u_row[:], I["urow"][:])
            nc.sync.dma_start(gidx_t[:], I["gidx"][:])
            nc.sync.dma_start(wcat[:], I["wcat"][:].rearrange("p (k m) -> p k m", k=6))
            nc.sync.dma_start(wout[:], I["wout"][:].rearrange("p (k m) -> p k m", k=6))
            nc.sync.dma_start(brow[:], I["brow"][:])

        ones8 = const.tile([1, GPC], fp32)
        nc.vector.memset(ones8[:], 1.0)
        onescol = const.tile([128, 1], bf16)
        nc.vector.memset(onescol[:], 1.0)

        # vec column offsets
        VO = {}
        off = 0
        for name, w_ in [("b1", 3), ("b2", 3), ("bp1", 3), ("bp2", 3),
                         ("bcv", 3), ("bcat", 6),
                         ("g0", 3), ("be0", 3), ("g1", 3), ("be1", 3),
                         ("g4", 3), ("be4", 3), ("g5", 3), ("be5", 3),
                         ("g6", 3), ("be6", 3),
                         ("gc0", 6), ("bc0", 6), ("gc1", 6), ("bc1", 6)]:
            VO[name] = (off, w_)
            off += w_
        def vcol(name):
            o, w_ = VO[name]
            return vecs[:, o:o + w_]

        # deg^-1/2
        dinv = const.tile([128, NW], fp32)
        nc.scalar.sqrt(dinv[:], deg_t[:])
        nc.vector.reciprocal(dinv[:], dinv[:])

        def tap(name, ap):
            if name not in taps:
                return
            t_ = nc.dram_tensor(f"tap_{name}", list(ap.shape), ap.dtype,
                                kind="ExternalOutput")
            tap_outs[name] = t_
            nc.sync.dma_start(t_[:], ap)

        def finish_early(src_ap, width=GPC):
            fin0 = spool.tile([C, GPC], fp32, name="fin0", tag="fin")
            nc.vector.memset(fin0[:], 0.0)
            nc.vector.tensor_scalar(fin0[:, 0:width], src_ap, 1.0, None, ALU.mult)
            nc.sync.dma_start(outT[:], fin0[:])

        for _rep in range(rep):
            # ---------------- helpers
            def gather_stats(st, width, tag):
                """st [128, width] per-core sums -> AllGather + local sum."""
                cin = dram.tile([128, width], fp32, tag=tag + "_ci")
                cout = dram.tile([NCORES * 128, width], fp32, tag=tag + "_co")
                nc.sync.dma_start(cin[:], st)
                nc.gpsimd.collective_compute(
                    "AllGather", ALU.bypass, replica_groups=grp,
                    ins=[cin[:]], outs=[cout[:]])
                allst = spool.tile([128, NCORES, width], fp32, tag=tag + "_as")
                nc.sync.dma_start(
                    allst[:], cout[:].rearrange("(g p) c -> p g c", g=NCORES))
                av = allst[:].rearrange("p g c -> p (g c)")
                red = spool.tile([128, width], fp32, tag=tag + "_rd")
                half = spool.tile([128, 4 * width], fp32, tag=tag + "_hf")
                nc.vector.tensor_tensor(
                    half[:], av[:, 0:4 * width], av[:, 4 * width:8 * width], ALU.add)
                nc.vector.tensor_tensor(
                    half[:, 0:2 * width], half[:, 0:2 * width],
                    half[:, 2 * width:4 * width], ALU.add)
                nc.vector.tensor_tensor(
                    red[:], half[:, 0:width], half[:, width:2 * width], ALU.add)
                return red

            def bn_coeffs(red, m_chunks, count, gname, bname, tag):
                """red [128, 2*m]: per-feature sum/sumsq -> gp, bp [128, m]
                gp = g/sigma, bp = b - mu*g/sigma."""
                gp = spool.tile([128, m_chunks], fp32, tag=tag + "_gp")
                bp = spool.tile([128, m_chunks], fp32, tag=tag + "_bp")
                mu = spool.tile([128, m_chunks], fp32, tag=tag + "_mu")
                var = spool.tile([128, m_chunks], fp32, tag=tag + "_va")
                inv_n = 1.0 / count
                sview = red.rearrange("p (m two) -> p m two", two=2)
                nc.vector.tensor_scalar(mu[:], sview[:, :, 0], inv_n, None, ALU.mult)
                nc.vector.tensor_scalar(var[:], sview[:, :, 1], inv_n, None, ALU.mult)
                musq = spool.tile([128, m_chunks], fp32, tag=tag + "_ms")
                nc.vector.tensor_mul(musq[:], mu[:], mu[:])
                nc.vector.tensor_tensor(var[:], var[:], musq[:], ALU.subtract)
                nc.vector.tensor_scalar(var[:], var[:], EPS, None, ALU.add)
                nc.scalar.sqrt(var[:], var[:])
                nc.vector.reciprocal(var[:], var[:])          # 1/sigma
                nc.vector.tensor_mul(gp[:], vcol(gname), var[:])
                nc.vector.tensor_mul(bp[:], gp[:], mu[:])
                nc.vector.tensor_tensor(bp[:], vcol(bname), bp[:], ALU.subtract)
                return gp, bp

            def fold_bn(wsrc, gp, bp, bias_name, tag):
                """Fold BN (gp, bp) into next layer: W' = diag(gp)W bf16,
                bias' = bp@W + b_next [128, 3] fp32."""
                wp = const.tile([128, 3, D], bf16, name=f"Wf_{tag}", tag=f"Wf_{tag}")
                for k in range(3):
                    nc.vector.tensor_scalar(
                        wp[:, k, :], wsrc[:, k, :], gp[:, k:k + 1], None, ALU.mult)
                bpb = spool.tile([128, 3], bf16, tag=tag + "_bpb")
                nc.vector.tensor_copy(bpb[:], bp[:])
                ps_b = typs.tile([128, GPC], fp32, tag="tiny")
                for m in range(3):
                    for k in range(3):
                        nc.tensor.matmul(
                            ps_b[:, m:m + 1],
                            lhsT=wsrc[:, k, m * 128:(m + 1) * 128],
                            rhs=bpb[:, k:k + 1], start=(k == 0), stop=(k == 2))
                bias = spool.tile([128, 3], fp32, tag=tag + "_bi")
                nc.vector.tensor_tensor(
                    bias[:], ps_b[:, 0:3], vcol(bias_name), ALU.add)
                return wp, bias

            def mlp_layer(x_t, w_t, bias_ap, tag, bias_is_tile=False):
                """x_t [128,3,NPC] bf16 -> relu(x@W + b) bf16 [128,3,NPC]
                + [128, 6] sum/sumsq stats via activation accum."""
                out = big.tile([128, 3, NPC], bf16, tag="big")
                sums = spool.tile([128, 3, NCH], fp32, tag=tag + "_sc")
                sqs = spool.tile([128, 3, NCH], fp32, tag=tag + "_qc")
                # j-outer so output COLUMNS complete early: the y-build
                # transposes (and other column consumers) can then overlap
                # this layer's later chunks instead of waiting for m=2.
                for j in range(NCH):
                    for m in range(3):
                        bcol = bias_ap[:, m:m + 1]
                        sl = slice(j * NCOL, (j + 1) * NCOL)
                        ps = mmps.tile([128, NCOL], fp32, tag="mm")
                        for k in range(3):
                            nc.tensor.matmul(
                                ps[:], lhsT=w_t[:, k, m * 128:(m + 1) * 128],
                                rhs=x_t[:, k, sl], start=(k == 0), stop=(k == 2))
                        nc.scalar.activation(out[:, m, sl], ps[:], AF.Relu,
                                             bias=bcol,
                                             accum_out=sums[:, m, j:j + 1])
                        sq = sqpool.tile([128, NCOL], fp32, name="sqscr", tag="sqscr")
                        if m == 0:   # balance sumsq work across Act and DVE
                            nc.scalar.activation(sq[:], out[:, m, sl], AF.Square,
                                                 accum_out=sqs[:, m, j:j + 1])
                        else:
                            nc.vector.tensor_mul(sq[:], out[:, m, sl], out[:, m, sl])
                            nc.vector.reduce_sum(sqs[:, m, j:j + 1], sq[:], axis=AX.X)
                st = spool.tile([128, 6], fp32, tag=tag + "_st")
                for m in range(3):
                    nc.vector.reduce_sum(st[:, 2 * m:2 * m + 1], sums[:, m, :], axis=AX.X)
                    nc.vector.reduce_sum(st[:, 2 * m + 1:2 * m + 2], sqs[:, m, :], axis=AX.X)
                return out, st

            # ---------------- pre-MLPs (xT_t loaded in the const section)
            x1, st1 = mlp_layer(xT_t, W["w1"], vcol("b1"), "l1")
            red1 = gather_stats(st1[:], 6, "ar1")
            gp1, bp1 = bn_coeffs(red1, 3, N, "g0", "be0", "bn1")
            w2f, bias2 = fold_bn(W["w2"], gp1, bp1, "b2", "f2")
            tap("x1", x1[:])

            x2, st2 = mlp_layer(x1, w2f, bias2, "l2", bias_is_tile=True)
            # st2 is stats of RAW r2 (pre-BN2) — exactly what BN2 needs.
            tap("x2", x2[:])
            if stage <= 2:
                red2d = gather_stats(st2[:], 6, "ar2")
                finish_early(red2d[0:C, 0:6], width=6)
                continue

            # ---------------- y = r2 * dinv (node-major bf16) ; AllGather
            # ybuf kept in SBUF for the self-loop PSUM seeds.
            ybuf = const.tile([128, NW, D], bf16)
            y_slice = dram.tile([NPC, D], bf16, tag="y_slice")
            for w in range(NW):
                for m in range(3):
                    trp = trps.tile([128, 128], bf16, tag="tr")
                    nc.tensor.transpose(
                        trp[:], x2[:, m, w * 128:(w + 1) * 128], ident[:])
                    nc.vector.tensor_scalar(
                        ybuf[:, w, m * 128:(m + 1) * 128], trp[:],
                        dinv[:, w:w + 1], None, ALU.mult)
                nc.sync.dma_start(y_slice[w * 128:(w + 1) * 128, :], ybuf[:, w, :])

            y_full = dram.tile([N, D], bf16, tag="y_full", addr_space="Shared")
            nc.gpsimd.collective_compute(
                "AllGather", ALU.bypass, replica_groups=grp,
                ins=[y_slice[:]], outs=[y_full[:]])
            load_deferred(ybuf[0:1, NW - 1, 0:1])

            # BN2 stats sync + folds overlap with the AllGather / gathers.
            red2 = gather_stats(st2[:], 6, "ar2")
            gp2, bp2 = bn_coeffs(red2, 3, N, "g1", "be1", "bn2")
            # only the weight scaling is folded for the conv: the bp2@Wc term
            # enters scaled per-node by u_d (tWc path below), not uniformly.
            wcf = const.tile([128, 3, D], bf16, name="Wf_fc", tag="Wf_fc")
            for k in range(3):
                nc.vector.tensor_scalar(
                    wcf[:, k, :], W["wc"][:, k, :], gp2[:, k:k + 1], None, ALU.mult)
            biasc = vcol("bcv")
            # tWc row [3, 128] fp32: (bp2 @ Wc) per m-chunk, transposed
            bpb2 = spool.tile([128, 3], bf16, tag="bpb2")
            nc.vector.tensor_copy(bpb2[:], bp2[:])
            ps_t = typs.tile([128, GPC], fp32, tag="tiny")
            for m in range(3):
                for k in range(3):
                    nc.tensor.matmul(
                        ps_t[:, m:m + 1],
                        lhsT=W["wc"][:, k, m * 128:(m + 1) * 128],
                        rhs=bpb2[:, k:k + 1], start=(k == 0), stop=(k == 2))
            twc_col = spool.tile([128, 3], fp32, tag="twc_c")
            nc.vector.tensor_copy(twc_col[:], ps_t[:, 0:3])
            twcT = spool.tile([1, 3 * 128], fp32, tag="twcT")
            for m in range(3):
                # per-column transpose so every PSUM read starts at partition 0
                # (walrus rejects partition-offset reads)
                trp_t = tfps.tile([128, 128], fp32, tag="trf")
                nc.tensor.transpose(trp_t[0:1, :], twc_col[:, m:m + 1], identf[:])
                nc.vector.tensor_copy(twcT[:, m * 128:(m + 1) * 128],
                                      trp_t[0:1, :])
            if stage <= 3:
                yck = spool.tile([128, D], bf16, name="yck", tag="yck")
                nc.sync.dma_start(yck[:], y_full[0:128, :])
                finish_early(yck[0:C, 0:GPC])
                continue

            # ---------------- sentence branch (emitted here to hide under
            # the AllGather): H_sentT [128, 6, GPC]
            HsT = spool.tile([128, 6, GPC], fp32, tag="HsT")
            for b in range(GPC):
                ps_ht = typs.tile([128, GPC], fp32, tag="tiny")
                hts = []
                for hsrc in (I["lasth"], I["firsth"]):
                    for sc in range(4):
                        ht = hspool.tile([128, E], bf16, name="ht", tag="ht")
                        if b == 0:
                            # gate behind y-build so these transfers hide
                            # under the AllGather instead of delaying L1
                            nc.vector.tensor_copy(ht[0:1, 0:1],
                                                  ybuf[0:1, NW - 1, 0:1])
                        nc.sync.dma_start(
                            ht[:], hsrc[b * S + sc * 128:b * S + (sc + 1) * 128, :])
                        hts.append(ht)
                for m in range(6):
                    for i, ht in enumerate(hts):
                        nc.tensor.matmul(
                            ps_ht[:, m:m + 1],
                            lhsT=ht[:, m * 128:(m + 1) * 128],
                            rhs=onescol[:],
                            start=(i == 0), stop=(i == 7))
                nc.vector.tensor_scalar(
                    HsT[:, :, b], ps_ht[:, 0:6],
                    1.0 / (2 * S), None, ALU.mult)
            tap("hsT", HsT[:])

            # ---------------- conv: per dst window, gather + selector matmul
            # into node-major PSUM A; then Wc' after aggregation.
            convT = big.tile([128, 3, NPC], bf16, tag="big")
            csums = spool.tile([128, 3, NW], fp32, tag="cv_sc")
            csqs = spool.tile([128, 3, NW], fp32, tag="cv_qc")
            for w in range(NW):
                ps_c = cvps.tile([128, D], fp32, tag="cv")
                # self-loop seed: A += I @ ybuf[w]
                nc.tensor.matmul(ps_c[:], lhsT=ident[:], rhs=ybuf[:, w, :],
                                 start=True, stop=False)
                nt = tpw[w]
                t0 = tstart[w]
                # balanced chunk sizes: a trailing 1-tile gather stalls the
                # DMA pipeline (desc-gen 1.3us > its own transfer time)
                nchunks = cdiv(nt, GCHUNK)
                base, extra = divmod(nt, nchunks)
                csizes = [base + (1 if i < extra else 0) for i in range(nchunks)]
                done = 0
                for cn in csizes:
                    gt = gpool.tile([128, GCHUNK * D], bf16, tag="g")
                    nc.gpsimd.dma_gather(
                        out_ap=gt[:, :cn * D].rearrange("p (t f) -> p t f", f=D),
                        in_ap=y_full[:],
                        idxs_ap=idx_t[:, (t0 + done) * 8:(t0 + done + cn) * 8],
                        num_idxs=cn * 128, num_idxs_reg=cn * 128, elem_size=D)
                    gv = gt[:, :cn * D].rearrange("p (t f) -> p t f", f=D)
                    for tl in range(cn):
                        tg = t0 + done + tl
                        sel = selp.tile([128, 128], bf16, tag="sel")
                        nc.vector.tensor_scalar(sel[:], iota[:], dst_t[:, tg:tg + 1],
                                                None, ALU.is_equal)
                        last = (done + tl == nt - 1)
                        nc.tensor.matmul(ps_c[:], lhsT=sel[:], rhs=gv[:, tl, :],
                                         start=False, stop=last)
                    done += cn
                # aggS = A * dinv[dst] (scale per partition) on Act
                aggS = spool.tile([128, D], bf16, tag="aggS")
                nc.scalar.activation(aggS[:], ps_c[:], AF.Copy,
                                     scale=dinv[:, w:w + 1])
                # transpose to feature-major, then Wc' + u*tWc + bias, relu
                aggT = spool.tile([128, 3, 128], bf16, tag="aggT")
                for m in range(3):
                    trp = trps.tile([128, 128], bf16, tag="tr")
                    nc.tensor.transpose(trp[:], aggS[:, m * 128:(m + 1) * 128],
                                        ident[:])
                    nc.scalar.activation(aggT[:, m, :], trp[:], AF.Copy)
                for m in range(3):
                    ps2f = mmps.tile([128, NCOL], fp32, tag="mm")
                    ps2 = ps2f[:, 0:128]
                    for k in range(3):
                        nc.tensor.matmul(
                            ps2[:], lhsT=wcf[:, k, m * 128:(m + 1) * 128],
                            rhs=aggT[:, k, :], start=(k == 0), stop=False)
                    nc.tensor.matmul(
                        ps2[:], lhsT=twcT[:, m * 128:(m + 1) * 128],
                        rhs=u_row[:, w * 128:(w + 1) * 128],
                        start=False, stop=True)
                    nc.scalar.activation(convT[:, m, w * 128:(w + 1) * 128],
                                         ps2[:], AF.Relu,
                                         bias=biasc[:, m:m + 1],
                                         accum_out=csums[:, m, w:w + 1])
                    sq = sqpool.tile([128, NCOL], fp32, name="sqscr2", tag="sqscr")
                    nc.vector.tensor_mul(sq[:, 0:128],
                                         convT[:, m, w * 128:(w + 1) * 128],
                                         convT[:, m, w * 128:(w + 1) * 128])
                    nc.vector.reduce_sum(csqs[:, m, w:w + 1], sq[:, 0:128], axis=AX.X)
            if stage <= 3.9:
                finish_early(convT[0:C, 0, 0:GPC])
                continue
            st4 = spool.tile([128, 6], fp32, tag="st4")
            for m in range(3):
                nc.vector.reduce_sum(st4[:, 2 * m:2 * m + 1], csums[:, m, :], axis=AX.X)
                nc.vector.reduce_sum(st4[:, 2 * m + 1:2 * m + 2], csqs[:, m, :], axis=AX.X)
            tap("convT", convT[:])
            red4 = gather_stats(st4[:], 6, "ar4")
            gp4, bp4 = bn_coeffs(red4, 3, N, "g4", "be4", "bn4")
            wp1f, biasp1 = fold_bn(W["wp1"], gp4, bp4, "bp1", "f4")
            if stage <= 4:
                finish_early(convT[0:C, 0, 0:GPC])
                continue

            # ---------------- post MLPs
            p1, st5 = mlp_layer(convT, wp1f, biasp1, "l5", bias_is_tile=True)
            red5 = gather_stats(st5[:], 6, "ar5")
            gp5, bp5 = bn_coeffs(red5, 3, N, "g5", "be5", "bn5")
            wp2f, biasp2 = fold_bn(W["wp2"], gp5, bp5, "bp2", "f5")

            # post2: row-major bf16 to DRAM (pre-BN6); stats via accum
            p2_dram = dram.tile([NPC, D], bf16, tag="p2")
            p2sc_s = spool.tile([128, 3, NCH], fp32, tag="p2s")
            p2sc_q = spool.tile([128, 3, NCH], fp32, tag="p2q")
            for j in range(NCH):
                sl = slice(j * NCOL, (j + 1) * NCOL)
                p2c = spool.tile([128, 3, NCOL], bf16, tag="p2c")
                for m in range(3):
                    ps = mmps.tile([128, NCOL], fp32, tag="mm")
                    for k in range(3):
                        nc.tensor.matmul(
                            ps[:], lhsT=wp2f[:, k, m * 128:(m + 1) * 128],
                            rhs=p1[:, k, sl], start=(k == 0), stop=(k == 2))
                    nc.scalar.activation(p2c[:, m, :], ps[:], AF.Relu,
                                         bias=biasp2[:, m:m + 1],
                                         accum_out=p2sc_s[:, m, j:j + 1])
                    sq = sqpool.tile([128, NCOL], fp32, name="sqscr3", tag="sqscr")
                    if m == 0:
                        nc.scalar.activation(sq[:], p2c[:, m, :], AF.Square,
                                             accum_out=p2sc_q[:, m, j:j + 1])
                    else:
                        nc.vector.tensor_mul(sq[:], p2c[:, m, :], p2c[:, m, :])
                        nc.vector.reduce_sum(p2sc_q[:, m, j:j + 1], sq[:], axis=AX.X)
                for nb in range(NCOL // 128):
                    rmw = spool.tile([128, D], bf16, tag="rmw")
                    for m in range(3):
                        trp = trps.tile([128, 128], bf16, tag="tr")
                        nc.tensor.transpose(
                            trp[:], p2c[:, m, nb * 128:(nb + 1) * 128], ident[:])
                        if m == 1:   # split PSUM->SBUF copies across engines
                            nc.scalar.activation(rmw[:, m * 128:(m + 1) * 128],
                                                 trp[:], AF.Copy)
                        else:
                            nc.vector.tensor_copy(rmw[:, m * 128:(m + 1) * 128],
                                                  trp[:])
                    nc.sync.dma_start(
                        p2_dram[j * NCOL + nb * 128:j * NCOL + (nb + 1) * 128, :],
                        rmw[:])
            st6 = spool.tile([128, 6], fp32, tag="st6")
            for m in range(3):
                nc.vector.reduce_sum(st6[:, 2 * m:2 * m + 1], p2sc_s[:, m, :], axis=AX.X)
                nc.vector.reduce_sum(st6[:, 2 * m + 1:2 * m + 2], p2sc_q[:, m, :], axis=AX.X)
            red6 = gather_stats(st6[:], 6, "ar6")
            gp6, bp6 = bn_coeffs(red6, 3, N, "g6", "be6", "bn6")
            if stage <= 5:
                finish_early(red6[0:C, 0:6], width=6)
                continue

            # ---------------- masked-node gather -> flT [128, 3, 16] bf16 (BN6'd)
            gth = spool.tile([128, D], bf16, tag="gth")
            nc.gpsimd.dma_gather(
                out_ap=gth[:].rearrange("p (t f) -> p t f", f=D),
                in_ap=p2_dram[:], idxs_ap=gidx_t[:],
                num_idxs=16, num_idxs_reg=16, elem_size=D)
            flT = spool.tile([128, 3, 16], bf16, tag="flT")
            for m in range(3):
                trp_full = trps.tile([128, 128], bf16, tag="tr")
                trp = trp_full[:, 0:16]
                nc.tensor.matmul(trp, lhsT=gth[0:16, m * 128:(m + 1) * 128],
                                 rhs=ident[0:16, 0:16], is_transpose=True)
                nc.vector.tensor_scalar(flT[:, m, :], trp,
                                        gp6[:, m:m + 1], bp6[:, m:m + 1],
                                        ALU.mult, ALU.add)
            tap("flT", flT[:])

            # ---------------- tail: outc, H_sent BN, att, out
            outcT = spool.tile([128, 6, GPC], fp32, tag="outcT")
            for m in range(6):
                ps_o = typs.tile([128, GPC], fp32, tag="tiny")
                for k in range(6):
                    kc, kj = k % 3, k // 3
                    nc.tensor.matmul(
                        ps_o[:], lhsT=wcat[:, k, m * 128:(m + 1) * 128],
                        rhs=flT[:, kc, kj::2], start=(k == 0), stop=(k == 5))
                nc.scalar.activation(outcT[:, m, :], ps_o[:], AF.Relu,
                                     bias=vcol("bcat")[:, m:m + 1])
            stt = spool.tile([128, 24], fp32, tag="stt")
            for m in range(6):
                nc.vector.reduce_sum(stt[:, 2 * m:2 * m + 1], outcT[:, m, :], axis=AX.X)
                sq = spool.tile([128, GPC], fp32, tag="ttsq")
                nc.scalar.square(sq[:], outcT[:, m, :])
                nc.vector.reduce_sum(stt[:, 2 * m + 1:2 * m + 2], sq[:], axis=AX.X)
                nc.vector.reduce_sum(stt[:, 12 + 2 * m:13 + 2 * m], HsT[:, m, :], axis=AX.X)
                nc.scalar.square(sq[:], HsT[:, m, :])
                nc.vector.reduce_sum(stt[:, 13 + 2 * m:14 + 2 * m], sq[:], axis=AX.X)
            redt = gather_stats(stt[:], 24, "art")
            gpc_, bpc_ = bn_coeffs(redt[:, 0:12], 6, B, "gc0", "bc0", "bnc")
            gph, bph = bn_coeffs(redt[:, 12:24], 6, B, "gc1", "bc1", "bnh")
            attT = spool.tile([128, 6, GPC], fp32, tag="attT")
            for m in range(6):
                nc.vector.tensor_scalar(attT[:, m, :], HsT[:, m, :],
                                        gph[:, m:m + 1], bph[:, m:m + 1],
                                        ALU.mult, ALU.add)
                nc.vector.tensor_scalar(outcT[:, m, :], outcT[:, m, :],
                                        gpc_[:, m:m + 1], bpc_[:, m:m + 1],
                                        ALU.mult, ALU.add)
                nc.vector.tensor_add(attT[:, m, :], attT[:, m, :], outcT[:, m, :])
            ps_ft = typs.tile([128, GPC], fp32, tag="tiny")
            ps_f = ps_ft[0:C, :]
            for k in range(6):
                nc.tensor.matmul(ps_f, lhsT=wout[:, k, :], rhs=attT[:, k, :],
                                 start=(k == 0), stop=False)
            nc.tensor.matmul(ps_f, lhsT=brow[0:1, D:D + C], rhs=ones8[:],
                             start=False, stop=True)
            fin = spool.tile([C, GPC], fp32, tag="fin")
            nc.vector.tensor_copy(fin[:], ps_f)
            nc.sync.dma_start(outT[:], fin[:])

    nc.compile()
    return nc, tap_outs


# ---------------------------------------------------------------- entry
_CACHE = {}


def _get_compiled(meta):
    key = meta
    if key not in _CACHE:
        nc, _ = build(meta)
        split_waits(nc)
        _CACHE[key] = nc
    return _CACHE[key]


def kernel(**inputs):
    in_maps, meta = preprocess(inputs)
    nc = _get_compiled(meta)
    from concourse import bass2jax
    results = bass2jax.run_bass_via_pjrt(nc, in_maps, n_cores=NCORES)
    out = np.concatenate([results[c]["outT"].T for c in range(NCORES)], axis=0)
    return out.astype(np.float32)
